# revision 53
# baseline (speedup 1.0000x reference)
"""Trainium2 Bass kernel for nn_Attention (cumulative masked softmax attention).

Reference computation:
    v   = tanh(x @ W + b)                  (B, T, F)
    a   = v . u                            (B, T)   -- query-independent logits
    e   = exp(a)[:, None, :] * tril * mask (B, T, T)
    alf = e / (sum_s e + EPS)
    c   = alf @ x                          (B, T, F)

Because the logits are query-independent and the mask is lower-triangular,
the (B,T,T) softmax-matmul collapses to a normalized running average along t:
    w[s]  = exp(a[s]) * mask[s]
    Z[t]  = EPS + cumsum(w)[t]
    c[t]  = (Z[t-1]/Z[t]) * c[t-1] + (w[t]/Z[t]) * x[t]
This linear recurrence maps 1:1 onto tensor_tensor_scan
(state = (data0 * state) + data1), which folds the softmax division into the
scan for free.

Layout: everything runs f-major ("transposed"): the kernel consumes ONLY
xT[b, f, t] (bf16, halving input DMA), computes vT = W^T x^T via PE matmuls
with W chunks stationary, alpha via tiny PE matmuls against u, broadcasts the
per-t row w across partitions via a PE rank-1 matmul, builds Z with a
broadcast prefix scan (one [128, T+1] tile per batch; inclusive/exclusive Z
are shifted slices of it), computes d0 = Z_ex/Z_in and wn = w/Z_in as wide
elementwise divides, multiplies and scans along the free (t) axis on
DVE/Pool, and DMAs out cT[b, f, t] f32. The host transposes the output back
to (B, T, F).

Sharding: data-parallel over batch B across 8 NeuronCores (2 batches/core).
"""

import numpy as np
import ml_dtypes

import concourse.bass as bass  # noqa: F401
import concourse.tile as tile
from concourse import bacc, mybir
from concourse.bass_utils import run_bass_kernel_spmd

B, T, F = 16, 1024, 512
EPS = 1e-7
NCORES = 8
B_LOC = B // NCORES          # batches per core
P = 128                      # partition tile
KC = F // P                  # f chunks (4)
NH = 2                       # t halves per batch
TH = T // NH                 # 512
NTB = T // P                 # t tiles per batch (8)
NTH = TH // P                # t tiles per half (4)

F32 = mybir.dt.float32
F32R = mybir.dt.float32r
BF16 = mybir.dt.bfloat16

ADD = mybir.AluOpType.add
MUL = mybir.AluOpType.mult
DIV = mybir.AluOpType.divide


def _build(have_b: bool, have_mask: bool, loop_n: int = 0):
    """Build the per-core Bass module. loop_n > 0 wraps the body in a
    hardware For_i loop (used only for timing)."""
    nc = bacc.Bacc("TRN2", target_bir_lowering=False, debug=False)

    xt_d = nc.dram_tensor("xT", [B_LOC, KC, P, T], BF16, kind="ExternalInput")
    cb_d = nc.dram_tensor("cb", [P, KC * F + KC], BF16, kind="ExternalInput")
    cr_d = nc.dram_tensor("cr", [P, 2 * P], F32R, kind="ExternalInput")
    if have_b:
        b_d = nc.dram_tensor("b", [P, KC], F32, kind="ExternalInput")
    if have_mask:
        m_d = nc.dram_tensor("m", [1, B_LOC, T], F32, kind="ExternalInput")
    c_d = nc.dram_tensor("cT", [B_LOC, KC, P, T], BF16, kind="ExternalOutput")

    Tanh = mybir.ActivationFunctionType.Tanh
    Copy = mybir.ActivationFunctionType.Copy
    Exp = mybir.ActivationFunctionType.Exp

    with tile.TileContext(nc) as tc:
        with (
            tc.tile_pool(name="const", bufs=1) as const,
            tc.tile_pool(name="xtp", bufs=1) as xtp,
            tc.tile_pool(name="vtp", bufs=3) as vtp,
            tc.tile_pool(name="smal", bufs=1) as smal,
            tc.tile_pool(name="rowp", bufs=1) as rowp,
            tc.tile_pool(name="ztp", bufs=1) as ztp,
            tc.tile_pool(name="dwp", bufs=1) as dwp,
            tc.tile_pool(name="ynp", bufs=6) as ynp,
            tc.tile_pool(name="ctp", bufs=1) as ctp,
            tc.tile_pool(name="ps_v", bufs=2, space="PSUM") as ps_v_pool,
            tc.tile_pool(name="ps_m", bufs=2, space="PSUM") as ps_m_pool,
            tc.tile_pool(name="ps_w", bufs=2, space="PSUM") as ps_w_pool,
        ):
            # ---- constants (packed; W k=0 chunk first for earliest start)
            cb_sb = const.tile([P, KC * F + KC], BF16)
            nc.sync.dma_start(out=cb_sb[:, 0:F], in_=cb_d.ap()[:, 0:F])
            cr_sb = const.tile([P, 2 * P], F32R)
            ones_sb = cr_sb[:, 0:P]
            eyer_sb = cr_sb[:, P:2 * P]
            zeroH = const.tile([P, TH], F32)
            nc.vector.memset(zeroH, 0.0)
            oneH = const.tile([P, TH], F32)
            nc.vector.memset(oneH, 1.0)
            if have_b:
                b_sb = const.tile([P, KC], F32)
                nc.scalar.dma_start(out=b_sb, in_=b_d.ap())
            if have_mask:
                m_sb = const.tile([1, B_LOC, T], F32)
                nc.gpsimd.dma_start(out=m_sb, in_=m_d.ap())

            # ---- input stream: one xT tile per batch, loaded per half ----
            xt_sb = {}
            for b in range(B_LOC):
                xt_sb[b] = xtp.tile([P, KC, T], BF16, name=f"xt{b}")
            # priority order: W_k0, xt(0,h0,k0), W_rest, xt(0,h0,k123), cr
            nc.sync.dma_start(
                out=xt_sb[0][:, 0, 0:TH],
                in_=xt_d.ap()[0].rearrange("k p t -> p k t")[:, 0, 0:TH])
            nc.sync.dma_start(out=cb_sb[:, F:], in_=cb_d.ap()[:, F:])
            nc.sync.dma_start(
                out=xt_sb[0][:, 1:KC, 0:TH],
                in_=xt_d.ap()[0].rearrange("k p t -> p k t")[:, 1:KC, 0:TH])
            nc.scalar.dma_start(out=cr_sb, in_=cr_d.ap())
            for b in range(B_LOC):
                for h in range(NH):
                    if b == 0 and h == 0:
                        continue
                    sl = slice(h * TH, (h + 1) * TH)
                    nc.sync.dma_start(
                        out=xt_sb[b][:, :, sl],
                        in_=xt_d.ap()[b].rearrange("k p t -> p k t")[:, :, sl])

            ct_sb = {}
            for b in range(B_LOC):
                ct_sb[b] = ctp.tile([P, KC, T], BF16, name=f"ct{b}")

            # per-batch tiles
            w_row = {}
            ztile = {}
            d0b = {}
            wnb = {}
            recb = {}
            for b in range(B_LOC):
                w_row[b] = rowp.tile([1, T], F32R, name=f"wrow{b}")
                # Z[:, 0] = EPS (exclusive zero prefix); scan fills 1..T
                ztile[b] = ztp.tile([P, T + 1], F32, name=f"zt{b}")
                d0b[b] = dwp.tile([P, T], F32, name=f"d0b{b}")
                wnb[b] = dwp.tile([P, T], BF16, name=f"wnb{b}")
                recb[b] = dwp.tile([P, T], F32, name=f"recb{b}")
                nc.vector.memset(ztile[b][:, 0:1], EPS)

            units = [(b, h) for b in range(B_LOC) for h in range(NH)]

            vT_t = {}

            def stage1_mm(ui, gp):
                """One g-pair of Phase A matmuls + its tanh."""
                b, h = units[ui]
                sl = slice(h * TH, (h + 1) * TH)
                if gp == 0:
                    vT_t[ui] = vtp.tile([P, KC, TH], BF16, name="vT")
                vT = vT_t[ui]
                ps_v = ps_v_pool.tile([P, 2, TH], F32, name="psv")
                for j in range(2):
                    g = 2 * gp + j
                    for k in range(KC):
                        nc.tensor.matmul(
                            ps_v[:, j, :],
                            cb_sb[:, k * F + g * P:k * F + (g + 1) * P],
                            xt_sb[b][:, k, sl],
                            start=(k == 0), stop=(k == KC - 1))
                _tanh_pair(ui, gp, ps_v)

            def _tanh_pair(ui, gp, ps_v):
                vT = vT_t[ui]
                if have_b:
                    for j in range(2):
                        g = 2 * gp + j
                        nc.scalar.activation(
                            out=vT[:, g, :], in_=ps_v[:, j, :],
                            func=Tanh, bias=b_sb[:, g:g + 1])
                else:
                    nc.scalar.activation(
                        out=vT[:, 2 * gp:2 * gp + 2, :], in_=ps_v,
                        func=Tanh)

            def stage1_mm_kouter(ui):
                """Unit-0 variant: k outermost so compute starts as soon as
                the k=0 slices of W and xT arrive."""
                b, h = units[ui]
                sl = slice(h * TH, (h + 1) * TH)
                vT_t[ui] = vtp.tile([P, KC, TH], BF16, name="vT")
                ps = [ps_v_pool.tile([P, 2, TH], F32, name="psv")
                      for _ in range(2)]
                for k in range(KC):
                    for gp in range(2):
                        for j in range(2):
                            g = 2 * gp + j
                            nc.tensor.matmul(
                                ps[gp][:, j, :],
                                cb_sb[:, k * F + g * P:k * F + (g + 1) * P],
                                xt_sb[b][:, k, sl],
                                start=(k == 0), stop=(k == KC - 1))
                for gp in range(2):
                    _tanh_pair(ui, gp, ps[gp])

            def stage1_alpha(ui):
                """alpha as a row: u^T @ vT, then exp straight into w_row."""
                b, h = units[ui]
                vT = vT_t[ui]
                ps_al = ps_m_pool.tile([1, TH], F32, name="psal", tag="al")
                for g in range(KC):
                    nc.tensor.matmul(
                        ps_al,
                        cb_sb[:, KC * F + g:KC * F + g + 1],
                        vT[:, g, :],
                        start=(g == 0), stop=(g == KC - 1))
                wsl = w_row[b][:, h * TH:(h + 1) * TH]
                nc.scalar.activation(out=wsl, in_=ps_al, func=Exp)
                if have_mask:
                    nc.vector.tensor_mul(
                        wsl, wsl, m_sb[:, b, h * TH:(h + 1) * TH])

            ps_wb_t = {}
            wb_sb_t = {}

            def stage2_bcast(ui, t0=None, tn=None):
                """broadcast w row across partitions (PE)."""
                b, h = units[ui]
                if t0 is None:
                    t0, tn = h * TH, TH
                ps_wb = ps_w_pool.tile([P, TH], F32, name="pswb")
                nc.tensor.matmul(ps_wb[:, 0:tn], ones_sb[0:1, :],
                                 w_row[b][:, t0:t0 + tn],
                                 start=True, stop=True)
                ps_wb_t[(ui, t0)] = ps_wb

            def stage2_zdiv(ui, t0=None, tn=None):
                """Z scan + reciprocal + d0/wn for one t-range."""
                b, h = units[ui]
                if t0 is None:
                    t0, tn = h * TH, TH
                ps_wb = ps_wb_t[(ui, t0)][:, 0:tn]
                z = ztile[b]
                init = EPS if t0 == 0 else z[:, t0:t0 + 1]
                nc.vector.tensor_tensor_scan(
                    out=z[:, 1 + t0:1 + t0 + tn],
                    data0=ps_wb, data1=zeroH[:, 0:tn], initial=init,
                    op0=ADD, op1=ADD)
                zin = z[:, 1 + t0:1 + t0 + tn]
                zex = z[:, t0:t0 + tn]
                rsl = recb[b][:, t0:t0 + tn]
                nc.vector.reciprocal(rsl, zin)
                wb_sb = vtp.tile([P, TH], BF16, name="wbsb", tag="wbsb")
                nc.scalar.copy(wb_sb[:, 0:tn], ps_wb)
                # wn first: it gates the yn multiply; d0 second
                nc.gpsimd.tensor_mul(
                    wnb[b][:, t0:t0 + tn], wb_sb[:, 0:tn], rsl)
                d0sl = d0b[b][:, t0:t0 + tn]
                nc.gpsimd.tensor_mul(d0sl, zex, rsl)

            def _bc(ap3, shape):
                return ap3.rearrange("p (o t) -> p o t", o=1).to_broadcast(
                    shape)

            def stage3_half(ui, t0=None, tn=None):
                """Per-k chained scans over one t-range."""
                b, h = units[ui]
                if t0 is None:
                    t0, tn = h * TH, TH
                sl = slice(t0, t0 + tn)
                yn = ynp.tile([P, KC, TH], BF16, name="ynh", tag="ynh")
                nc.vector.tensor_tensor(
                    out=yn[:, :, 0:tn], in0=xt_sb[b][:, :, sl],
                    in1=_bc(wnb[b][:, sl], (P, KC, tn)), op=MUL)
                ct = ct_sb[b]
                for k in range(KC):
                    init = 0.0 if t0 == 0 else ct[:, k, t0 - 1:t0]
                    nc.vector.tensor_tensor_scan(
                        out=ct[:, k, sl], data0=d0b[b][:, sl],
                        data1=yn[:, k, 0:tn], initial=init, op0=MUL, op1=ADD)
                if False:
                    pass
                else:
                    for kp in range(KC // 2):
                        nc.sync.dma_start(
                            out=c_d.ap()[b, 2 * kp:2 * kp + 2].rearrange(
                                "k p t -> p k t")[:, :, sl],
                            in_=ct[:, 2 * kp:2 * kp + 2, sl])

            import contextlib
            loop_ctx = (tc.For_i(0, loop_n, 1) if loop_n
                        else contextlib.nullcontext())
            with loop_ctx:
                # PE warmup on the first-loaded W chunk: keeps the clock
                # ramp going while xT streams in (results discarded).
                ps_warm = ps_w_pool.tile([P, TH], F32, name="pswb",
                                         tag="pswb")
                for _ in range(4):
                    nc.tensor.matmul(ps_warm, cb_sb[:, 0:P],
                                     cb_sb[:, 0:F], start=True, stop=True)
                # software pipeline: unit u's row/broadcast chain is threaded
                # through unit u+1's Phase A so the PE never stalls and
                # downstream engines get fed early. Batch 0 uses one
                # full-T block mid-pipeline; batch 1 scans per-half so the
                # tail chain is short.
                stage1_mm(0, 0)
                stage1_mm(0, 1)
                stage1_alpha(0)
                stage1_mm(1, 0)
                stage2_bcast(0)
                stage2_zdiv(0)
                stage1_mm(1, 1)
                stage1_alpha(1)
                stage1_mm(2, 0)
                stage2_bcast(1)
                stage2_zdiv(1)
                stage3_half(0)
                stage1_mm(2, 1)
                stage1_alpha(2)
                stage1_mm(3, 0)
                stage2_bcast(2)
                stage2_zdiv(2)
                stage3_half(1)
                stage1_mm(3, 1)
                stage1_alpha(3)
                stage2_bcast(3)
                stage2_zdiv(3)
                stage3_half(2)
                stage3_half(3)

    nc.compile()
    return nc


_NC_CACHE: dict = {}


def _get_nc(have_b, have_mask, loop_n=0):
    key = (have_b, have_mask, loop_n)
    if key not in _NC_CACHE:
        _NC_CACHE[key] = _build(have_b, have_mask, loop_n)
    return _NC_CACHE[key]


def make_core_maps(x, W, u, b=None, mask_f=None):
    """Build the 8 per-core input maps from full inputs."""
    bf16 = ml_dtypes.bfloat16
    # cb = [W chunks (k-major) | u] in bf16: cb[p, k*F+c] = W[k*P+p, c],
    # cb[p, KC*F+g] = u[g*P+p]
    cb = np.empty((P, KC * F + KC), dtype=np.float32)
    cb[:, :KC * F] = W.reshape(KC, P, F).transpose(1, 0, 2).reshape(P, KC * F)
    cb[:, KC * F:] = u.reshape(KC, P).T
    cb = cb.astype(bf16)
    # cr = [ones | eye] as f32 bits (f32r on device)
    cr = np.concatenate([np.ones((P, P), dtype=np.float32),
                         np.eye(P, dtype=np.float32)], axis=1)
    maps = []
    for core in range(NCORES):
        xs = x[core * B_LOC:(core + 1) * B_LOC]          # [B_LOC, T, F]
        xT = np.ascontiguousarray(xs.transpose(0, 2, 1)) # [B_LOC, F, T]
        xT = xT.reshape(B_LOC, KC, P, T).astype(bf16)
        m = {"xT": xT, "cb": cb, "cr": cr}
        if b is not None:
            m["b"] = np.ascontiguousarray(b.reshape(KC, P).T)
        if mask_f is not None:
            m["m"] = np.ascontiguousarray(
                mask_f[core * B_LOC:(core + 1) * B_LOC].reshape(
                    1, B_LOC, T))
        maps.append(m)
    return maps


def kernel(x, mask, W, b, u):
    x = np.asarray(x, dtype=np.float32)
    W = np.asarray(W, dtype=np.float32)
    b = np.asarray(b, dtype=np.float32)
    u = np.asarray(u, dtype=np.float32)
    mask_f = np.asarray(mask).astype(np.float32)

    have_b = bool(np.any(b != 0.0))
    have_mask = bool(np.any(mask_f != 1.0))

    nc = _get_nc(have_b, have_mask)
    in_maps = make_core_maps(x, W, u,
                             b if have_b else None,
                             mask_f if have_mask else None)
    res = run_bass_kernel_spmd(nc, in_maps, core_ids=list(range(NCORES)))
    outs = []
    for r in res.results:
        cT = r["cT"].reshape(B_LOC, F, T).astype(np.float32)  # [b, f, t]
        outs.append(np.ascontiguousarray(cT.transpose(0, 2, 1)))
    return np.concatenate(outs, axis=0).astype(np.float32)
